# revision 19
# baseline (speedup 1.0000x reference)
"""Trainium2 Bass kernel for nn_Mixture (moe_routing).

Model (B=8192, D=1024, K=8 experts, H=2048):
  1. Hard k-means routing: cluster[b] = argmin_k ||x_b - c_k||^2
  2. Per-expert MLP head: lls[b] = tanh(x_b @ W1[e] + b1[e]) @ W2[e] + b2[e],
     e = cluster[b]  (reference computes densely for all K; we compute
     sparsely, only the routed expert per sample -> ~K x less compute).

Two SPMD launches on 8 NeuronCores:
  Launch A (routing, data-parallel over B): each core computes
    d2'[s,k] = ||c_k||^2 - 2 x_s.c_k. Scores use an exact bf16 hi/lo split
    (x = xh+xl, c = ch+cl; products xh.ch + xh.cl + xl.ch accumulated in
    fp32 on the PE) -> ~1e-4 score error vs a minimum argmin gap of ~1.5e-2
    in this data (plain bf16 flips 13 samples, fp32r flips 1 -> not safe).
    Centroids are the stationary operand (k-major, N=512 moving), scores
    are PE-transposed back to sample-major, then a DVE argmin with
    first-min tie-break matches jnp.argmin exactly.
  Host: pads each expert group to a multiple of 128 and packs the
    resulting "subtiles" into a per-core uniform slot template (same
    compiled program for all cores; per-core differences are pure data:
    which expert's weights land in each weight-slot input).
  Launch B (expert MLP, balanced expert-parallel): per core, for each
    128-sample subtile: x_sub @ W1[slot] (bf16 operands, fp32 accumulate,
    ~216ns/matmul warm = PE roofline), tanh on ACT, then fused
    multiply+reduce against W2 on DVE (scalar_tensor_tensor accum_out).

Perf notes baked into the structure: PE warm-up matmuls during the DMA
ramp (HAM clock gate), partition-packed DRAM layouts so DMA descriptors
are multi-KB, outputs PE-transposed so the store is a few contiguous
descriptors (a scattered store's completion semaphore otherwise trails
by ~10ns/descriptor), and weight/x DMAs fine-grained on the sync queue
so matmuls chase tiles as they land.
"""

import math
import os
import sys

import numpy as np

B, D, K, H = 8192, 1024, 8, 2048
NCORES = 8
SUB = 128  # subtile: samples per matmul M-tile
SHARD = B // NCORES  # samples per core in routing launch

_CONCOURSE_READY = False
_ROUTING_CACHE = {}
_MLP_CACHE = {}
TRACE_DIR = None  # test harness may set this to capture a profile
LAST_RESULTS = {}  # launch name -> BassKernelResults (for the test harness)


def _run_spmd(name, nc, in_maps):
    from concourse.bass_utils import run_bass_kernel_spmd

    kw = {}
    if TRACE_DIR is not None:
        d = os.path.join(TRACE_DIR, name)
        os.makedirs(d, exist_ok=True)
        kw = dict(trace=True, tmpdir=d)
    res = run_bass_kernel_spmd(nc, in_maps, list(range(NCORES)), **kw)
    LAST_RESULTS[name] = res
    return res


def _ensure_concourse():
    """Make concourse importable + install the NTFF profile hook glue."""
    global _CONCOURSE_READY
    if _CONCOURSE_READY:
        return
    for p in ("/root/.axon_site", "/root/.axon_site/_ro/trn_rl_repo",
              "/root/.axon_site/_ro/pypackages"):
        if os.path.isdir(p) and p not in sys.path:
            sys.path.append(p)

    # bass_utils wants antenv.axon_hooks for trace=True under axon; the
    # container ships a stub antenv without it. Provide the glue module.
    if "antenv.axon_hooks" not in sys.modules:
        import types
        mod = types.ModuleType("antenv.axon_hooks")
        _hook_box = [None]
        mod.set_axon_ntff_profile_hook = lambda h: _hook_box.__setitem__(0, h)
        mod.get_axon_ntff_profile_hook = lambda: _hook_box[0]
        sys.modules["antenv.axon_hooks"] = mod

        so_path = "/opt/axon/libaxon_pjrt.so"
        if os.path.exists(so_path):
            import contextlib
            import ctypes
            try:
                lib = ctypes.CDLL(so_path)
                if hasattr(lib, "axon_start_nrt_profile"):
                    lib.axon_start_nrt_profile.argtypes = [
                        ctypes.POINTER(ctypes.c_int64), ctypes.c_size_t]
                    lib.axon_start_nrt_profile.restype = ctypes.c_int64
                    lib.axon_stop_nrt_profile.argtypes = [ctypes.c_char_p]
                    lib.axon_stop_nrt_profile.restype = ctypes.c_int64

                    @contextlib.contextmanager
                    def _hook(output_dir, device_ids):
                        import jax
                        jax.devices()
                        if device_ids:
                            ids = (ctypes.c_int64 * len(device_ids))(*device_ids)
                            rc = lib.axon_start_nrt_profile(ids, len(device_ids))
                        else:
                            rc = lib.axon_start_nrt_profile(None, 0)
                        if rc != 0:
                            raise RuntimeError(f"axon_start_nrt_profile rc={rc}")
                        try:
                            yield
                        finally:
                            n = lib.axon_stop_nrt_profile(str(output_dir).encode())
                            if n <= 0:
                                print(f"ntff profile: {n} files written",
                                      file=sys.stderr)

                    mod.set_axon_ntff_profile_hook(_hook)
            except OSError:
                pass

    import concourse.bass_utils as bu
    # Artifact upload needs a fish bucket; irrelevant here.
    bu.upload_artifacts = lambda tmpdir: "local://noupload"
    _CONCOURSE_READY = True


# ---------------------------------------------------------------------------
# Launch A: routing
# ---------------------------------------------------------------------------

def _build_routing():
    """argmin_k(||c_k||^2 - 2 x.c_k) for a SHARD of samples.

    Scores use an exact bf16 hi/lo split: x = xh + xl, c = ch + cl (each
    bf16), stacked along the contraction dim -> one bf16 matmul chain over
    2D rows reproduces the fp32 product to ~2^-17, at full bf16 PE rate.
    Min argmin gap in the data is ~1.5e-2, error here is ~1e-4."""
    import concourse.bacc as bacc
    import concourse.tile as tile
    from concourse import mybir
    from concourse.masks import make_identity

    f32 = mybir.dt.float32
    bf16 = mybir.dt.bfloat16

    DC = D // 128           # 8 chunks per half
    DC2 = 2 * DC            # 16 total (hi block then lo block)
    ST = SHARD // SUB       # sample tiles per core

    nc = bacc.Bacc("TRN2", target_bir_lowering=False, debug=False)
    # partition-packed layout: row p holds chunk-major contiguous data, so
    # each DMA descriptor covers kilobytes instead of one 2KB line
    xT2 = nc.dram_tensor("xT2", [128, DC2 * SHARD], bf16,
                         kind="ExternalInput").ap()
    # centroids partition-packed on host: [128, DC2*K] (256B/partition,
    # 128 packets; the old [2D, K] rearrange was 2048 16-byte packets that
    # clogged the DMA queue ahead of x)
    ct2 = nc.dram_tensor("ct2", [128, DC2 * K], bf16, kind="ExternalInput").ap()
    cl = nc.dram_tensor("cl", [SHARD], f32, kind="ExternalOutput").ap()

    with tile.TileContext(nc) as tc:
        import contextlib
        with contextlib.ExitStack() as ctx:
            const = ctx.enter_context(tc.tile_pool(name="const", bufs=1))
            xpool = ctx.enter_context(tc.tile_pool(name="xpool", bufs=1))
            work = ctx.enter_context(tc.tile_pool(name="work", bufs=4))
            psum = ctx.enter_context(tc.tile_pool(name="psum", bufs=1, space="PSUM"))
            outp = ctx.enter_context(tc.tile_pool(name="outp", bufs=1))

            # PE warm-up first in program order so it only waits on the DVE
            # memsets (which run right after the entry barrier): ramps the
            # HAM clock to full before the real matmul stream.
            warm_sb = const.tile([128, 512], bf16)
            nc.vector.memset(warm_sb[:], 0.0)
            warm_w = const.tile([128, 128], bf16)
            nc.vector.memset(warm_w[:], 0.0)
            warm_ps = psum.tile([128, 512], f32, tag="sh0", name="warm_ps")
            for _ in range(10):
                nc.tensor.matmul(warm_ps[:], warm_w[:], warm_sb[:],
                                 start=True, stop=True)

            # centroids first (tiny), then x tiles (the bulk)
            ct_sb = const.tile([128, DC2, K], bf16)
            nc.sync.dma_start(out=ct_sb[:], in_=ct2)

            # x in geometric group DMAs: small first pieces so the matmul
            # stream starts chasing early, coarse later pieces so the queue
            # issues few dma_starts (each costs ~0.7us of queue time)
            XGROUPS = [(0, 2), (2, 2), (4, 4), (8, 4), (12, 4)]
            xt_g = []
            for g, (c0, nchunk) in enumerate(XGROUPS):
                t = xpool.tile([128, nchunk, SHARD], bf16, tag=f"xg{g}",
                               name=f"xg{g}")
                nc.sync.dma_start(
                    out=t[:],
                    in_=xT2[:, c0 * SHARD:(c0 + nchunk) * SHARD])
                xt_g.append(t)

            def xt_slice(d, lo, hi):
                for g, (c0, nchunk) in enumerate(XGROUPS):
                    if c0 <= d < c0 + nchunk:
                        return xt_g[g][:, d - c0, lo:hi]
                raise AssertionError(d)

            ident = const.tile([128, 128], f32)
            make_identity(nc, ident[:])

            iota_i = const.tile([128, K], mybir.dt.int32)
            nc.gpsimd.iota(iota_i[:], pattern=[[1, K]], base=0, channel_multiplier=0)
            # k - 100: (d2p==min)*(k-100) is minimized at the FIRST min k
            # (ties: smaller k wins; non-matches give 0 > k-100)
            iota_m = const.tile([128, K], f32)
            nc.vector.tensor_scalar(out=iota_m[:], in0=iota_i[:],
                                    scalar1=-100.0, scalar2=None,
                                    op0=mybir.AluOpType.add)

            # ||c_k||^2 from the reconstructed c = ch + cl (exact in f32)
            ceff = work.tile([128, DC, K], f32, name="ceff")
            nc.vector.tensor_tensor(out=ceff[:], in0=ct_sb[:, 0:DC, :],
                                    in1=ct_sb[:, DC:DC2, :],
                                    op=mybir.AluOpType.add)
            sq = work.tile([128, DC, K], f32, name="sq")
            nc.vector.tensor_tensor(out=sq[:], in0=ceff[:], in1=ceff[:],
                                    op=mybir.AluOpType.mult)
            ccp = work.tile([128, K], f32, name="ccp")
            nc.vector.tensor_reduce(out=ccp[:], in_=sq.rearrange("p a k -> p k a"),
                                    axis=mybir.AxisListType.X, op=mybir.AluOpType.add)
            ccb = work.tile([128, K], f32, name="ccb")
            import concourse.bass_isa as bass_isa
            nc.gpsimd.partition_all_reduce(ccb[:], ccp[:], channels=128,
                                           reduce_op=bass_isa.ReduceOp.add)
            # ||c||^2 replicated 4x along the batch dim for the batched
            # argmin chain
            ccb4 = work.tile([128, 4, K], f32, name="ccb4")
            for t in range(4):
                nc.vector.tensor_copy(out=ccb4[:, t, :], in_=ccb[:])

            cl_tile = outp.tile([128, ST], f32)
            clT_out = [outp.tile([4, 128], f32, tag=f"clT{b}", name=f"clT{b}")
                       for b in range(ST // 4)]

            # products: xh.ch + xh.cl + xl.ch (xl.cl ~1e-4, dropped).
            # x tiles: 0..DC-1 = xh chunks, DC..2DC-1 = xl chunks;
            # ct chunks: 0..DC-1 = ch, DC..2DC-1 = cl.
            # k-major matmuls: centroids stationary (8-col LDWEIGHTS is ~free,
            # vs 192 full LDWs sample-major), samples moving at N=512.
            # ordered by x-chunk arrival: both ch and cl products for each
            # xh chunk as it lands, xl products last (the final x tile then
            # gates only two matmuls instead of sixteen)
            pairs = []
            for d in range(DC):
                pairs += [(d, d), (d, DC + d)]
            for d in range(DC):
                pairs.append((DC + d, d))
            SH2 = SHARD // 512
            pss = [psum.tile([K, 512], f32, tag=f"sh{h}", name=f"sh{h}")
                   for h in range(SH2)]
            for i, (xd, cd) in enumerate(pairs):
                for h in range(SH2):
                    nc.tensor.matmul(pss[h][:], ct_sb[:, cd, :],
                                     xt_slice(xd, h * 512, (h + 1) * 512),
                                     start=(i == 0), stop=(i == len(pairs) - 1))
            sc_sb = []
            for h in range(SH2):
                t = work.tile([K, 512], f32, tag=f"scsb{h}", name=f"scsb{h}")
                nc.vector.tensor_copy(out=t[:], in_=pss[h][:])
                sc_sb.append(t)
            # argmin in batches of 4 sample-tiles: one transpose psum tile
            # per batch, then a mostly-batched DVE chain (8 ops per 4 tiles
            # instead of 24), with the transposed output written and DMA'd
            # incrementally per batch.
            for b in range(ST // 4):
                trp4 = psum.tile([128, 4, K], f32, tag="tr", bufs=2,
                                 name=f"trp4_{b}")
                for t in range(4):
                    s = 4 * b + t
                    h, off = divmod(s * SUB, 512)
                    nc.tensor.transpose(trp4[:, t, :],
                                        sc_sb[h][:, off:off + SUB],
                                        ident[0:K, 0:K])
                # d2' = cc - 2*scores  (batched over 4 tiles)
                d2p4 = work.tile([128, 4, K], f32, tag="d2p", name="d2p")
                nc.vector.scalar_tensor_tensor(out=d2p4[:], in0=trp4[:],
                                               scalar=-2.0,
                                               in1=ccb4[:],
                                               op0=mybir.AluOpType.mult,
                                               op1=mybir.AluOpType.add)
                mcol4 = work.tile([128, 4], f32, tag="mcol", name="mcol")
                nc.vector.tensor_reduce(out=mcol4[:], in_=d2p4[:],
                                        axis=mybir.AxisListType.X,
                                        op=mybir.AluOpType.min)
                cand4 = work.tile([128, 4, K], f32, tag="cand", name="cand")
                for t in range(4):
                    nc.vector.scalar_tensor_tensor(
                        out=cand4[:, t, :], in0=d2p4[:, t, :],
                        scalar=mcol4[:, t:t + 1], in1=iota_m[:],
                        op0=mybir.AluOpType.is_equal,
                        op1=mybir.AluOpType.mult)
                amin4 = work.tile([128, 4], f32, tag="amin", name="amin")
                nc.vector.tensor_reduce(out=amin4[:], in_=cand4[:],
                                        axis=mybir.AxisListType.X,
                                        op=mybir.AluOpType.min)
                nc.vector.tensor_scalar(out=cl_tile[:, 4 * b:4 * b + 4],
                                        in0=amin4[:], scalar1=100.0,
                                        scalar2=None, op0=mybir.AluOpType.add)
                # transpose this batch's 4 columns -> contiguous output DMA
                clT_ps = psum.tile([4, 128], f32, tag="tr", bufs=2,
                                   name=f"clT_ps{b}")
                nc.tensor.transpose(clT_ps[:], cl_tile[:, 4 * b:4 * b + 4],
                                    ident[:])
                nc.vector.tensor_copy(out=clT_out[b][:], in_=clT_ps[:])
                nc.sync.dma_start(
                    out=cl.rearrange("(t p) -> t p", p=128)[4 * b:4 * b + 4],
                    in_=clT_out[b][:])

    nc.compile()
    return nc


def _hilo_T(a):
    """[N, D] f32 -> [2D, N] bf16: rows 0..D-1 = hi(a.T), D..2D-1 = lo(a.T)."""
    import ml_dtypes
    af = a.astype(np.float32)
    hi = af.astype(ml_dtypes.bfloat16)
    lo = (af - hi.astype(np.float32)).astype(ml_dtypes.bfloat16)
    out = np.empty((2 * a.shape[1], a.shape[0]), dtype=ml_dtypes.bfloat16)
    out[:a.shape[1]] = hi.T
    out[a.shape[1]:] = lo.T
    return out


def _pack_rows(a):
    """[C*128, M] -> [128, C*M]: row p = concat over chunks c of a[c*128+p].
    Makes each SBUF partition's DMA source bytes contiguous."""
    C = a.shape[0] // 128
    return np.ascontiguousarray(
        a.reshape(C, 128, a.shape[1]).transpose(1, 0, 2).reshape(128, -1))


def _run_routing(x, centroids):
    if "nc" not in _ROUTING_CACHE:
        _ROUTING_CACHE["nc"] = _build_routing()
    nc = _ROUTING_CACHE["nc"]

    ct2 = _pack_rows(_hilo_T(centroids))
    in_maps = []
    for i in range(NCORES):
        in_maps.append({"xT2": _pack_rows(_hilo_T(x[i * SHARD:(i + 1) * SHARD])),
                        "ct2": ct2})
    res = _run_spmd("routing", nc, in_maps)
    cluster = np.concatenate([res.results[i]["cl"] for i in range(NCORES)])
    return np.rint(cluster).astype(np.int32)


# ---------------------------------------------------------------------------
# Host: balanced packing of expert groups into a uniform slot template
# ---------------------------------------------------------------------------

def _templates(cap):
    """Descending compositions of cap into <=4 parts, fewest parts first."""
    out = []

    def rec(rem, mx, cur):
        if rem == 0:
            out.append(tuple(cur))
            return
        if len(cur) == 4:
            return
        for t in range(min(mx, rem), 0, -1):
            rec(rem - t, t, cur + [t])

    rec(cap, cap, [])
    out.sort(key=lambda p: (len(p), -p[0]))
    return out


def _try_pack(tmpl, need):
    """Assign slot pieces (8 per template position) to experts so every
    expert's subtile need is covered. Returns {(pos, copy): expert}."""
    avail = {p: 8 for p in range(len(tmpl))}
    assign = {}
    order = sorted(range(len(need)), key=lambda e: -need[e])
    for e in order:
        rem = need[e]
        while rem > 0:
            # largest piece with size <= rem, else smallest piece >= rem
            cands = [p for p in avail if avail[p] > 0]
            if not cands:
                return None
            le = [p for p in cands if tmpl[p] <= rem]
            if le:
                p = max(le, key=lambda p: tmpl[p])
            else:
                p = min(cands, key=lambda p: tmpl[p])
            avail[p] -= 1
            assign[(p, avail[p])] = e
            rem -= tmpl[p]
    return assign


def _make_plan(counts):
    """Choose template + per-core slot->expert plan for the actual counts."""
    need = [(c + SUB - 1) // SUB for c in counts]
    total = max(1, sum(need))
    base = (total + NCORES - 1) // NCORES
    for cap in range(base, base + 8):
        for tmpl in _templates(cap):
            a = _try_pack(tmpl, need)
            if a is not None:
                return tmpl, a
    raise RuntimeError(f"no packing found for counts={counts}")


# ---------------------------------------------------------------------------
# Launch B: expert MLP
# ---------------------------------------------------------------------------

def _build_mlp(tmpl, with_b1):
    import concourse.bacc as bacc
    import concourse.tile as tile
    from concourse import mybir

    f32 = mybir.dt.float32
    bf16 = mybir.dt.bfloat16
    m = len(tmpl)
    cap_sub = sum(tmpl)          # subtiles per core
    cap = cap_sub * SUB          # samples per core
    DC = D // 128                # 8 contraction chunks
    HC = H // 512                # 4 H chunks of 512

    # subtile index -> slot position
    slot_of = []
    for p, t in enumerate(tmpl):
        slot_of += [p] * t

    nc = bacc.Bacc("TRN2", target_bir_lowering=False, debug=False)
    # partition-packed layouts (row p = chunk-major contiguous; see _pack_rows)
    xgT = nc.dram_tensor("xgT", [128, DC * cap], bf16, kind="ExternalInput").ap()
    wslots = [nc.dram_tensor(f"wslot{j}", [128, DC * H], bf16,
                             kind="ExternalInput").ap()
              for j in range(m)]
    w2s = nc.dram_tensor("w2s", [m, H], f32, kind="ExternalInput").ap()
    b2s = nc.dram_tensor("b2s", [m], f32, kind="ExternalInput").ap()
    if with_b1:
        b1s = nc.dram_tensor("b1s", [m, H], f32, kind="ExternalInput").ap()
    y = nc.dram_tensor("y", [cap], f32, kind="ExternalOutput").ap()

    import concourse.bass as bass
    from concourse.masks import make_identity

    def bcast_ap(src_ap, parts=128):
        return bass.AP(tensor=src_ap.tensor, offset=src_ap.offset,
                       ap=[[0, parts]] + list(src_ap.ap))

    with tile.TileContext(nc) as tc:
        import contextlib
        with contextlib.ExitStack() as ctx:
            const = ctx.enter_context(tc.tile_pool(name="const", bufs=1))
            xpool = ctx.enter_context(tc.tile_pool(name="xpool", bufs=1))
            wpool = ctx.enter_context(tc.tile_pool(name="wpool", bufs=1))
            hpool = ctx.enter_context(tc.tile_pool(name="hpool", bufs=4))
            spool = ctx.enter_context(tc.tile_pool(name="spool", bufs=4))
            ppool = ctx.enter_context(tc.tile_pool(name="ppool", bufs=6))
            psum = ctx.enter_context(tc.tile_pool(name="psum", bufs=2, space="PSUM"))
            outp = ctx.enter_context(tc.tile_pool(name="outp", bufs=1))

            # PE warm-up first in program order: only waits on the DVE
            # memsets, so it starts right after the entry barrier and ramps
            # the HAM clock while the first DMAs are in flight.
            warm_sb = const.tile([128, 512], bf16)
            nc.vector.memset(warm_sb[:], 0.0)
            warm_w = const.tile([128, 128], bf16)
            nc.vector.memset(warm_w[:], 0.0)
            warm_ps = psum.tile([128, 512], f32, tag="ps0", name="warm_ps")
            for _ in range(12):
                nc.tensor.matmul(warm_ps[:], warm_w[:], warm_sb[:],
                                 start=True, stop=True)

            # Critical-path DMAs on the Sync HWDGE queue in consumption
            # order: slot-0 weight pieces interleaved with x groups (the
            # matmul stream chases these). Later slots go coarse (16KB
            # runs) on the Scalar HWDGE queue, off the critical queue.
            w_sb = {}

            def load_slot_coarse(j):
                # 2 pieces of 16KB-per-partition runs, on the sync queue
                # BEHIND the critical w0/x stream (a parallel queue halves
                # the critical stream's bandwidth - measured regression)
                per = DC // 2
                tiles = []
                for pc in range(2):
                    t = wpool.tile([128, per, H], bf16, tag=f"w{j}_{pc}",
                                   name=f"w{j}_{pc}")
                    nc.sync.dma_start(
                        out=t[:],
                        in_=wslots[j][:, pc * per * H:(pc + 1) * per * H])
                    tiles.append(t)

                def slot_slice(d, lo, hi, tiles=tiles, per=per):
                    pc, dd = divmod(d, per)
                    return tiles[pc][:, dd, lo:hi]
                w_sb[j] = slot_slice

            # x is packed SUBTILE-major on the host ([t][d][128 cols]), so
            # each piece delivers complete subtiles in demand order. w0
            # pieces (chunk-by-chunk) interleave with the early x pieces:
            # subtile 0's d-major walk chases w0 chunk arrival; subtiles
            # 1+ find their x already resident.
            j0 = slot_of[0]
            XS = [(0, 1), (1, 1), (2, 2), (4, cap_sub - 4)]
            xs_tiles = []
            w0_tiles = []

            def emit_xs(i):
                t0, nt = XS[i]
                t = xpool.tile([128, nt, DC, SUB], bf16, tag=f"xs{i}",
                               name=f"xs{i}")
                nc.sync.dma_start(
                    out=t[:], in_=xgT[:, t0 * DC * SUB:(t0 + nt) * DC * SUB])
                xs_tiles.append(t)

            emit_xs(0)
            for pc in range(8):
                t = wpool.tile([128, 1, H], bf16, tag=f"w{j0}_{pc}",
                               name=f"w{j0}_{pc}")
                nc.sync.dma_start(
                    out=t[:], in_=wslots[j0][:, pc * H:(pc + 1) * H])
                w0_tiles.append(t)
                if pc in (0, 2, 4):
                    emit_xs({0: 1, 2: 2, 4: 3}[pc])

            def w0_slice(d, lo, hi):
                return w0_tiles[d][:, 0, lo:hi]
            w_sb[j0] = w0_slice

            def xt_sub(t_i, d):
                for i, (t0, nt) in enumerate(XS):
                    if t0 <= t_i < t0 + nt:
                        return xs_tiles[i][:, t_i - t0, d, :]
                raise AssertionError(t_i)

            # prefetch the other slots immediately, behind the critical
            # w0/x stream on the same queue
            for jn in range(m):
                if jn not in w_sb:
                    load_slot_coarse(jn)

            # W2 rows to one partition, then replicate via GpSimd (idle engine,
            # avoids 128x DMA write amplification of a stride-0 broadcast DMA)
            w2row = const.tile([1, m, H], f32)
            nc.scalar.dma_start(out=w2row[:], in_=bass.AP(
                tensor=w2s.tensor, offset=w2s.offset, ap=[[0, 1]] + list(w2s.ap)))
            w2rep = []
            for j in range(m):
                t = const.tile([128, H], f32, tag=f"w2rep{j}", name=f"w2rep{j}")
                nc.gpsimd.partition_broadcast(t[:], w2row[:, j, :])
                w2rep.append(t)
            # b2 to one partition + broadcast (a bcast DMA would be 128
            # tiny packets on the critical queue)
            b2row = const.tile([1, m], f32)
            nc.scalar.dma_start(out=b2row[:], in_=bass.AP(
                tensor=b2s.tensor, offset=b2s.offset, ap=[[0, 1]] + list(b2s.ap)))
            b2b = const.tile([128, m], f32)
            nc.gpsimd.partition_broadcast(b2b[:], b2row[:])
            b1rep = []
            if with_b1:
                b1row = const.tile([1, m, H], f32)
                nc.scalar.dma_start(out=b1row[:], in_=bass.AP(
                    tensor=b1s.tensor, offset=b1s.offset, ap=[[0, 1]] + list(b1s.ap)))
                for j in range(m):
                    t = const.tile([128, H], f32, tag=f"b1rep{j}", name=f"b1rep{j}")
                    nc.gpsimd.partition_broadcast(t[:], b1row[:, j, :])
                    b1rep.append(t)

            ident = const.tile([128, 128], f32)
            make_identity(nc, ident[:])

            ytile = outp.tile([128, cap_sub], f32)
            yTa = outp.tile([cap_sub - 1, 128], f32)
            yTb = outp.tile([1, 128], f32)

            for t_i in range(cap_sub):
                j = slot_of[t_i]
                wt = w_sb[j]

                # d-major walks chase the chunk-by-chunk w0 DMA arrival.
                # The LAST subtile goes hc-major with its final H-chunk
                # split 2x256, so the end-of-stream ACT/DVE drain is only
                # a 256-wide chain.
                last = t_i == cap_sub - 1
                if not last:
                    hchunks = [(hc * 512, 512, f"ps{hc}") for hc in range(HC)]
                    pss = [psum.tile([128, n], f32, tag=tag, name=tag)
                           for (_, n, tag) in hchunks]
                    for d in range(DC):
                        lhs = xt_sub(t_i, d)
                        for ci, (off, n, _) in enumerate(hchunks):
                            nc.tensor.matmul(pss[ci][:], lhs,
                                             wt(d, off, off + n),
                                             start=(d == 0), stop=(d == DC - 1))
                else:
                    hchunks = [(0, 512, "ps0"), (512, 512, "ps1"),
                               (1024, 512, "ps2"), (1536, 256, "ps3"),
                               (1792, 256, "ps3")]
                    pss = [psum.tile([128, n], f32, tag=tag, name=f"{tag}_{ci}")
                           for ci, (_, n, tag) in enumerate(hchunks)]
                    for ci, (off, n, _) in enumerate(hchunks):
                        for d in range(DC):
                            lhs = xt_sub(t_i, d)
                            nc.tensor.matmul(pss[ci][:], lhs,
                                             wt(d, off, off + n),
                                             start=(d == 0), stop=(d == DC - 1))

                partials = ppool.tile([128, len(hchunks)], f32, tag="partials",
                                      name="partials")
                for ci, (off, n, _) in enumerate(hchunks):
                    if with_b1:
                        nc.vector.tensor_tensor(
                            out=pss[ci][:], in0=pss[ci][:],
                            in1=b1rep[j][:, off:off + n],
                            op=mybir.AluOpType.add)
                    th = hpool.tile([128, n], f32, tag="th", name="th")
                    nc.scalar.activation(out=th[:], in_=pss[ci][:],
                                         func=mybir.ActivationFunctionType.Tanh)
                    # partials[:, ci] = sum_free(th * w2_chunk)
                    scratch = spool.tile([128, n], f32, tag="scr", name="scr")
                    nc.vector.scalar_tensor_tensor(
                        out=scratch[:], in0=th[:], scalar=1.0,
                        in1=w2rep[j][:, off:off + n],
                        op0=mybir.AluOpType.mult, op1=mybir.AluOpType.mult,
                        accum_out=partials[:, ci:ci + 1])
                ysum = ppool.tile([128, 1], f32, tag="ysum", name="ysum")
                nc.vector.tensor_reduce(out=ysum[:], in_=partials[:],
                                        axis=mybir.AxisListType.X,
                                        op=mybir.AluOpType.add)
                nc.vector.tensor_scalar(out=ytile[:, t_i:t_i + 1], in0=ysum[:],
                                        scalar1=b2b[:, j:j + 1], scalar2=None,
                                        op0=mybir.AluOpType.add)

                # transpose + store the first cap_sub-1 columns while the
                # last subtile is still computing; only the final column's
                # tiny transpose remains on the tail
                if t_i == cap_sub - 2:
                    yTa_ps = psum.tile([cap_sub - 1, 128], f32, tag="ps0",
                                       name="yTa_ps")
                    nc.tensor.transpose(yTa_ps[:], ytile[:, 0:cap_sub - 1],
                                        ident[:])
                    nc.vector.tensor_copy(out=yTa[:], in_=yTa_ps[:])
                    nc.sync.dma_start(
                        out=y.rearrange("(t p) -> t p", p=128)[0:cap_sub - 1],
                        in_=yTa[:])

            yTb_ps = psum.tile([1, 128], f32, tag="ps1", name="yTb_ps")
            nc.tensor.transpose(yTb_ps[:], ytile[:, cap_sub - 1:cap_sub],
                                ident[:])
            nc.vector.tensor_copy(out=yTb[:], in_=yTb_ps[:])
            nc.sync.dma_start(
                out=y.rearrange("(t p) -> t p", p=128)[cap_sub - 1:cap_sub],
                in_=yTb[:])

    nc.compile()
    return nc, cap, cap_sub


def _run_mlp(x, W1, b1, W2, b2, cluster):
    import ml_dtypes

    counts = np.bincount(cluster, minlength=K)
    tmpl, assign = _make_plan(list(counts))
    with_b1 = bool(np.any(b1 != 0.0))
    m = len(tmpl)

    key = (tmpl, with_b1)
    if key not in _MLP_CACHE:
        _MLP_CACHE[key] = _build_mlp(tmpl, with_b1)
    nc, cap, cap_sub = _MLP_CACHE[key]

    # Expert index queues (padded with -1 to a multiple of SUB)
    queues = {}
    for e in range(K):
        idx = np.nonzero(cluster == e)[0]
        pad = (-len(idx)) % SUB
        queues[e] = np.concatenate([idx, -np.ones(pad, dtype=np.int64)])
    qpos = {e: 0 for e in range(K)}

    # piece (pos, copy) -> core: copy c of position p goes to core c.
    # assign maps (pos, copy) -> expert; unassigned copies are idle.
    core_slot_expert = [[None] * m for _ in range(NCORES)]
    core_samp = [np.full(cap, -1, dtype=np.int64) for _ in range(NCORES)]
    sub_base = np.cumsum([0] + list(tmpl))  # subtile offset of each slot
    for (p, cpy), e in assign.items():
        core = cpy  # one copy of each position per core
        core_slot_expert[core][p] = e
        want = tmpl[p] * SUB
        take = queues[e][qpos[e]:qpos[e] + want]
        qpos[e] += len(take)
        s0 = sub_base[p] * SUB
        core_samp[core][s0:s0 + len(take)] = take
    for e in range(K):
        assert qpos[e] >= np.count_nonzero(queues[e] >= 0), \
            f"expert {e} not fully covered"

    xf = x.astype(np.float32)
    zero_w = np.zeros((128, D // 128 * H), dtype=ml_dtypes.bfloat16)
    wpack_cache = {}

    def packed_w(e):
        if e not in wpack_cache:
            wpack_cache[e] = _pack_rows(W1[e].astype(ml_dtypes.bfloat16))
        return wpack_cache[e]

    in_maps = []
    for c in range(NCORES):
        samp = core_samp[c]
        mask = samp >= 0
        xg = np.zeros((cap, D), dtype=np.float32)
        xg[mask] = xf[samp[mask]]
        # subtile-major pack: [p][subtile][chunk][col] so x DMA pieces
        # deliver complete subtiles in the order the matmuls consume them
        xt = np.ascontiguousarray(xg.T).astype(ml_dtypes.bfloat16)
        xpk = (xt.reshape(D // 128, 128, cap // SUB, SUB)
               .transpose(1, 2, 0, 3).reshape(128, -1))
        im = {
            "xgT": np.ascontiguousarray(xpk),
            "w2s": np.zeros((m, H), dtype=np.float32),
            "b2s": np.zeros((m,), dtype=np.float32),
        }
        if with_b1:
            im["b1s"] = np.zeros((m, H), dtype=np.float32)
        for p in range(m):
            e = core_slot_expert[c][p]
            if e is None:
                im[f"wslot{p}"] = zero_w
            else:
                im[f"wslot{p}"] = packed_w(e)
                im["w2s"][p] = W2[e]
                im["b2s"][p] = b2[e]
                if with_b1:
                    im["b1s"][p] = b1[e]
        in_maps.append(im)

    res = _run_spmd("mlp", nc, in_maps)

    out = np.zeros(B, dtype=np.float32)
    for c in range(NCORES):
        samp = core_samp[c]
        mask = samp >= 0
        yc = res.results[c]["y"]
        out[samp[mask]] = yc[mask]
    return out, res


def kernel(x, centroids, W1, b1, W2, b2):
    _ensure_concourse()
    x = np.asarray(x)
    centroids = np.asarray(centroids)
    W1 = np.asarray(W1)
    b1 = np.asarray(b1)
    W2 = np.asarray(W2)
    b2 = np.asarray(b2)

    cluster = _run_routing(x, centroids)
    out, _ = _run_mlp(x, W1, b1, W2, b2, cluster)
    return out

